# revision 32
# baseline (speedup 1.0000x reference)
"""Trainium2 Bass kernel for nn_DecoderHead (MAE-style decoder head), v2.

Strategy (8 NeuronCores): data-parallel over batch B=4 x 2-way token split
per batch. Each core computes Q/K/V only for its OWN 1024 tokens; K and V^T
halves are exchanged per head via 2-rank AllGather (pipelined 3 heads ahead
of the attention consumer, so the collective is fully hidden). Keys are laid
out in RANK ORDER (attention is key-permutation invariant), so the AllGather
result reads back with plain direct DMAs - no indirect gather needed.

On-device layout is feature-major (x^T: [D, T_own] with D on partitions).
Input scatter and output transpose are done host-side. Heads padded 96->128;
a ones-row injected in V (via bias) makes PV emit softmax denominators for
free. LN gamma/beta are folded into adjacent weights host-side. LN mean/rstd
and softmax-denominator broadcasts run on the (otherwise idle) GPSIMD engine
via partition_broadcast instead of PE matmuls. All GEMMs run in bf16 (fp8
e4m3 was measured at >2e-2 rel err - over the harness gate). PSUM is
organized as 2-bank pair tiles (3x "a" + 1x "v") so evictions are 1024 wide.
"""

import sys
import numpy as np

sys.path.insert(0, "/opt/trn_rl_repo")

import ml_dtypes

P = 128
B = 4
N_VIS = 512
T = 2048          # N_TOT
D = 768
KD = D // P       # 6
NH = 8
DH = 96
HID = 3072
HB = HID // P     # 24
DEPTH = 2
TQ = 1024         # own tokens per core
CH = 512          # token chunk
EPS = 1e-5

BF16 = ml_dtypes.bfloat16

_cache = {}


def _build():
    import concourse.bass as bass
    import concourse.mybir as mybir
    import concourse.tile as tile
    from concourse import bacc
    from concourse.masks import make_identity

    dt = mybir.dt
    nc = bacc.Bacc("TRN2", target_bir_lowering=False, debug=False, num_devices=8)

    x0 = nc.dram_tensor("x0", (KD, P, TQ), dt.float16, kind="ExternalInput").ap()
    wqkv = nc.dram_tensor("wqkv", (DEPTH, D, NH * 3 * P), dt.bfloat16, kind="ExternalInput").ap()
    bqkv = nc.dram_tensor("bqkv", (DEPTH, P, NH * 3), dt.float32, kind="ExternalInput").ap()
    sqkv = nc.dram_tensor("sqkv", (DEPTH, P, NH * 3), dt.float32, kind="ExternalInput").ap()
    wo = nc.dram_tensor("wo", (DEPTH, NH * P, D), dt.bfloat16, kind="ExternalInput").ap()
    bwo = nc.dram_tensor("bwo", (DEPTH, P, KD), dt.float32, kind="ExternalInput").ap()
    w1 = nc.dram_tensor("w1", (DEPTH, D, HID), dt.bfloat16, kind="ExternalInput").ap()
    b1 = nc.dram_tensor("b1", (DEPTH, P, HB), dt.float32, kind="ExternalInput").ap()
    s1 = nc.dram_tensor("s1", (DEPTH, P, HB), dt.float32, kind="ExternalInput").ap()
    w2 = nc.dram_tensor("w2", (DEPTH, HID, D), dt.bfloat16, kind="ExternalInput").ap()
    b2 = nc.dram_tensor("b2", (DEPTH, P, KD), dt.float32, kind="ExternalInput").ap()
    s2 = nc.dram_tensor("s2", (DEPTH, P, KD), dt.float32, kind="ExternalInput").ap()
    wdec = nc.dram_tensor("wdec", (D, D), dt.bfloat16, kind="ExternalInput").ap()
    bdec = nc.dram_tensor("bdec", (P, KD), dt.float32, kind="ExternalInput").ap()
    y = nc.dram_tensor("y", (KD, P, TQ), dt.float16, kind="ExternalOutput").ap()

    RG = [[0, 1], [2, 3], [4, 5], [6, 7]]

    with tile.TileContext(nc) as tc:
        from contextlib import ExitStack
        ctx = ExitStack()
        with ctx:
            const = ctx.enter_context(tc.tile_pool(name="const", bufs=1))
            xp = ctx.enter_context(tc.tile_pool(name="xp", bufs=1))
            xlnp = ctx.enter_context(tc.tile_pool(name="xlnp", bufs=1))
            kp = ctx.enter_context(tc.tile_pool(name="kp", bufs=4))
            vp = ctx.enter_context(tc.tile_pool(name="vp", bufs=4))
            qp = ctx.enter_context(tc.tile_pool(name="qp", bufs=2))
            kstp = ctx.enter_context(tc.tile_pool(name="kstp", bufs=2))
            vtmp = ctx.enter_context(tc.tile_pool(name="vtmp", bufs=2))
            vstp = ctx.enter_context(tc.tile_pool(name="vstp", bufs=2))
            ptp = ctx.enter_context(tc.tile_pool(name="ptp", bufs=2))
            aop = ctx.enter_context(tc.tile_pool(name="aop", bufs=1))
            hp = ctx.enter_context(tc.tile_pool(name="hp", bufs=1))
            yp = ctx.enter_context(tc.tile_pool(name="yp", bufs=1))
            wkvp = ctx.enter_context(tc.tile_pool(name="wkvp", bufs=2))
            wqp = ctx.enter_context(tc.tile_pool(name="wqp", bufs=2))
            wop = ctx.enter_context(tc.tile_pool(name="wop", bufs=1))
            w1p = ctx.enter_context(tc.tile_pool(name="w1p", bufs=5))
            w2p = ctx.enter_context(tc.tile_pool(name="w2p", bufs=4))
            scr = ctx.enter_context(tc.tile_pool(name="scr", bufs=2))
            xcp = ctx.enter_context(tc.tile_pool(name="xcp", bufs=2))
            statp = ctx.enter_context(tc.tile_pool(name="statp", bufs=1))
            biasp = ctx.enter_context(tc.tile_pool(name="biasp", bufs=1))

            drp = ctx.enter_context(tc.tile_pool(name="drp", bufs=1, space="DRAM"))
            # PSUM budget (8 banks): psA tag "a" 3x2-bank + tag "v" 1x2-bank
            psA = ctx.enter_context(tc.tile_pool(name="psA", bufs=3, space="PSUM"))

            f32 = dt.float32
            bf = dt.bfloat16
            f16 = dt.float16
            AF = mybir.ActivationFunctionType
            ALU = mybir.AluOpType

            ones_kx1b = const.tile([P, 1], bf, tag="ones_kx1b")
            nc.any.memset(ones_kx1b[:], 1.0 / D)
            ones_kx1h = const.tile([P, 1], f16, tag="ones_kx1h")
            nc.any.memset(ones_kx1h[:], 1.0 / D)
            sq11 = const.tile([1, 1], f32, tag="sq11")
            ident_bf = const.tile([P, P], bf, tag="ident_bf")
            make_identity(nc, ident_bf[:])
            eps_t = const.tile([1, 1], f32, tag="eps")
            nc.any.memset(eps_t[:], EPS)

            # persistent activations (own token half only)
            x = xp.tile([P, KD, TQ], f16, tag="x")
            xln = xlnp.tile([P, KD, TQ], bf, tag="xln")
            aout = aop.tile([P, NH, TQ], bf, tag="aout")

            # ---- PE warmup (runs during input DMA, releases HAM throttle) ----
            for wu in range(32):
                wt = psA.tile([P, 2, CH], bf, tag="a", name=f"wu{wu}")
                nc.tensor.transpose(wt[:, 0, 0:P], ident_bf[:], ident_bf[:])

            # ---- input DMA: x0^T own half ----
            for tb in range(2):
                for kt in range(KD):
                    nc.sync.dma_start(x[:, kt, tb * CH:(tb + 1) * CH],
                                      x0[kt, :, tb * CH:(tb + 1) * CH])

            _warm_ctr = [0]

            def layer_norm(chunks, out_fn=None):
                """LN over feature dim of x. Two passes: stats for ALL chunks
                first (PE ones-matmuls scaled by 1/D so PSUM holds mu/E2
                directly; sum chain interleaved with scalar-engine squares),
                then per-chunk postproc + gpsimd broadcast + DVE centering.
                Keep-warm transposes bridge the postproc PE idle window."""
                pss = {}
                for cs in chunks:
                    ps = psA.tile([P, 2, CH], f32, tag="a")
                    sqs = []
                    for kt in range(KD):
                        sq = scr.tile([P, CH], bf, tag="scrb")
                        nc.scalar.activation(sq[:], x[:, kt, cs:cs + CH], AF.Square)
                        sqs.append(sq)
                        nc.tensor.matmul(ps[0:1, 0, :], ones_kx1h[:], x[:, kt, cs:cs + CH],
                                         start=(kt == 0), stop=(kt == KD - 1))
                        if kt > 0:
                            nc.tensor.matmul(ps[0:1, 1, :], ones_kx1b[:], sqs[kt - 1][:],
                                             start=(kt == 1), stop=False)
                    nc.tensor.matmul(ps[0:1, 1, :], ones_kx1b[:], sqs[KD - 1][:],
                                     start=False, stop=True)
                    pss[cs] = ps
                # prefetch the Sqrt ACT table while PE finishes stats
                nc.scalar.activation(sq11[:], eps_t[0:1, 0:1], AF.Sqrt)
                # HAM keep-warm fillers through the (idle-during-LN) "v" pair;
                # the "a" ring stays free for stats/next-phase chains
                for wv in range(12):
                    wt2 = psA.tile([P, 2, CH], bf, tag="v", bufs=1,
                                   name=f"lnw{_warm_ctr[0]}_{wv}")
                    nc.tensor.transpose(wt2[:, 0, 0:P], ident_bf[:], ident_bf[:])
                _warm_ctr[0] += 1
                for cs in chunks:
                    ps = pss[cs]
                    st = statp.tile([1, 4 * CH], f32, tag="stats")
                    sg = lambda i: st[0:1, i * CH:(i + 1) * CH]
                    stb = statp.tile([1, 2 * CH], bf, tag="stb")
                    pms = xcp.tile([P, 2, CH], bf, tag="pms")
                    # mu path first so centering-sub can start early
                    nc.vector.tensor_copy(stb[0:1, 0:CH], ps[0:1, 0, :])
                    nc.gpsimd.partition_broadcast(pms[:, 0:1, :], stb[0:1, 0:CH])
                    nc.vector.tensor_mul(sg(0), stb[0:1, 0:CH], stb[0:1, 0:CH])  # mu^2
                    nc.vector.tensor_sub(sg(1), ps[0:1, 1, :], sg(0))          # var
                    nc.scalar.activation(sg(0), sg(1), AF.Sqrt,
                                         bias=eps_t[0:1, 0:1])
                    nc.vector.reciprocal_approx_fast(sg(3), sg(0))             # r
                    nc.vector.tensor_copy(stb[0:1, CH:2 * CH], sg(3))
                    nc.gpsimd.partition_broadcast(pms[:, 1:2, :], stb[0:1, CH:2 * CH])
                    for kt in range(KD):
                        xc = xcp.tile([P, CH], bf, tag="xc")
                        eng = nc.vector
                        eng.tensor_sub(xc[:], x[:, kt, cs:cs + CH], pms[:, 0, :])
                        if out_fn is None:
                            eng.tensor_mul(xln[:, kt, cs:cs + CH], xc[:],
                                           pms[:, 1, :])
                        else:
                            out_fn(kt, cs, xc, pms[:, 1, :], eng)

            def kv_stage(l, h, bq, sq_t):
                """K/V chains for own half, V^T transpose, AllGather both."""
                wkv = wkvp.tile([P, KD, 2 * P], bf, tag="wkv")
                nc.sync.dma_start(
                    wkv[:], wqkv[l, :, h * 3 * P + P:(h + 1) * 3 * P].rearrange(
                        "(kt p) c -> p kt c", p=P))
                kst = kstp.tile([P, TQ], bf, tag="kst")
                vtm = vtmp.tile([P, TQ], bf, tag="vtm")
                kvi = drp.tile([P, 2 * TQ], bf, tag=f"kvi{l}_{h}", name=f"kvi{l}_{h}")
                kvo = drp.tile([2 * P, 2 * TQ], bf, tag=f"kvo{l}_{h}",
                               name=f"kvo{l}_{h}")
                for m, dst in ((1, kst), (2, vtm)):
                    ps = psA.tile([P, 2, CH], f32, tag="a")
                    for bank in range(2):
                        for kt in range(KD):
                            nc.tensor.matmul(ps[:, bank, :],
                                             wkv[:, kt, (m - 1) * P:m * P],
                                             xln[:, kt, bank * CH:(bank + 1) * CH],
                                             start=(kt == 0), stop=(kt == KD - 1))
                    c = h * 3 + m
                    nc.vector.tensor_scalar(dst[:, :], ps[:, 0:2, :],
                                            sq_t[:, c:c + 1], bq[:, c:c + 1],
                                            ALU.mult, ALU.add)
                    if m == 1:
                        # stage K immediately, split across DMA queues
                        nc.sync.dma_start(kvi[:, 0:CH], kst[:, 0:CH])
                        nc.sync.dma_start(kvi[:, CH:TQ], kst[:, CH:TQ])
                vst = vstp.tile([P, 8, P], bf, tag="vst")
                for i in range(8):
                    pst = psA.tile([P, 2, CH], bf, tag="a")
                    nc.tensor.transpose(pst[:, 0, 0:P], vtm[:, i * P:(i + 1) * P],
                                        ident_bf[:])
                    nc.vector.tensor_copy(vst[:, i, :], pst[:, 0, 0:P])
                nc.sync.dma_start(kvi[:, TQ:TQ + CH], vst[:, 0:4, :])
                nc.sync.dma_start(kvi[:, TQ + CH:2 * TQ], vst[:, 4:8, :])
                nc.gpsimd.collective_compute(
                    "AllGather", mybir.AluOpType.bypass, replica_groups=RG,
                    ins=[kvi.opt()], outs=[kvo.opt()])
                # rank-ordered readback: key block r*8..r*8+7 = rank r's tokens
                ktile = kp.tile([P, T], bf, tag="k")
                vaug = vp.tile([P, T // P, P], bf, tag="vaug")
                for r in range(2):
                    nc.gpsimd.dma_start(ktile[:, r * TQ:(r + 1) * TQ],
                                        kvo[r * P:(r + 1) * P, 0:TQ])
                    nc.gpsimd.dma_start(vaug[:, r * 8:(r + 1) * 8, :],
                                        kvo[r * P:(r + 1) * P, TQ:2 * TQ])
                return ktile, vaug

            def attn_stage(l, h, ktile, vaug, bq, sq_t):
                """Q chain + scores^T -> exp -> PV + denom normalize."""
                wq = wqp.tile([P, KD, P], bf, tag="wq")
                nc.sync.dma_start(
                    wq[:], wqkv[l, :, h * 3 * P:h * 3 * P + P].rearrange(
                        "(kt p) c -> p kt c", p=P))
                qt = qp.tile([P, TQ], bf, tag="q")
                ps = psA.tile([P, 2, CH], f32, tag="a")
                for bank in range(2):
                    for kt in range(KD):
                        nc.tensor.matmul(ps[:, bank, :], wq[:, kt, :],
                                         xln[:, kt, bank * CH:(bank + 1) * CH],
                                         start=(kt == 0), stop=(kt == KD - 1))
                c = h * 3
                nc.vector.tensor_scalar(qt[:, :], ps[:, 0:2, :],
                                        sq_t[:, c:c + 1], bq[:, c:c + 1],
                                        ALU.mult, ALU.add)
                if h == 0:
                    # bridge the first AllGather readback wait (HAM keep-warm)
                    for wv in range(8):
                        wt3 = psA.tile([P, 2, CH], bf, tag="v", bufs=1,
                                       name=f"aw{l}_{wv}")
                        nc.tensor.transpose(wt3[:, 0, 0:P], ident_bf[:],
                                            ident_bf[:])
                for cs in (0, CH):
                    pvc = psA.tile([P, 2, CH], f32, tag="v", bufs=1)
                    pv = pvc[:, 0, :]
                    nmm = 0
                    for half in range(2):
                        pt = ptp.tile([P, 8, CH], bf, tag="pt")
                        for tp in range(4):
                            tb = half * 8 + 2 * tp
                            sc = psA.tile([P, 2, CH], f32, tag="a")
                            for k2 in range(2):
                                nc.tensor.matmul(
                                    sc[:, k2, :],
                                    ktile[:, (tb + k2) * P:(tb + k2 + 1) * P],
                                    qt[:, cs:cs + CH],
                                    start=True, stop=True)
                            nc.scalar.activation(pt[:, 2 * tp:2 * tp + 2, :],
                                                 sc[:, 0:2, :], AF.Exp)
                            for k2 in range(2):
                                nc.tensor.matmul(pv, vaug[:, tb + k2, :],
                                                 pt[:, 2 * tp + k2, :],
                                                 start=(nmm == 0),
                                                 stop=(nmm == T // P - 1))
                                nmm += 1
                    # normalize by denominator (row 96 of pv)
                    dn = scr.tile([1, CH], f32, tag="scr")
                    nc.vector.tensor_copy(dn[:], pv[DH:DH + 1, :])
                    rc1 = statp.tile([1, CH], f32, tag="stb")
                    nc.vector.reciprocal_approx_fast(rc1[:], dn[:])
                    rcb = scr.tile([P, CH], f32, tag="scr")
                    nc.gpsimd.partition_broadcast(rcb[:], rc1[:])
                    nc.vector.tensor_mul(aout[:, h, cs:cs + CH], pv, rcb[:])

            for l in range(DEPTH):
                # ---------- LN1 (own half only) ----------
                layer_norm([0, CH])
                # prefetch Exp table on idle scalar before attention
                nc.scalar.activation(sq11[:], eps_t[0:1, 0:1], AF.Exp)

                # per-layer bias/scale tiles
                bq = biasp.tile([P, NH * 3], f32, tag="bq")
                nc.sync.dma_start(bq[:], bqkv[l])
                sq_t = biasp.tile([P, NH * 3], f32, tag="sq")
                nc.sync.dma_start(sq_t[:], sqkv[l])
                bo_t = biasp.tile([P, KD], f32, tag="bo")
                nc.sync.dma_start(bo_t[:], bwo[l])
                b1_t = biasp.tile([P, HB], f32, tag="b1")
                nc.sync.dma_start(b1_t[:], b1[l])
                s1_t = biasp.tile([P, HB], f32, tag="s1")
                nc.sync.dma_start(s1_t[:], s1[l])
                b2_t = biasp.tile([P, KD], f32, tag="b2")
                nc.sync.dma_start(b2_t[:], b2[l])
                s2_t = biasp.tile([P, KD], f32, tag="s2")
                nc.sync.dma_start(s2_t[:], s2[l])

                wot = wop.tile([P, NH, D], bf, tag="wo")
                nc.sync.dma_start(wot[:], wo[l].rearrange("(kb p) c -> p kb c", p=P))

                # ---------- attention: kv pipeline 4 ahead of consumer ----------
                kv_tiles = {}
                for h in range(4):
                    kv_tiles[h] = kv_stage(l, h, bq, sq_t)
                for h in range(NH):
                    attn_stage(l, h, *kv_tiles.pop(h), bq, sq_t)
                    if h + 4 < NH:
                        kv_tiles[h + 4] = kv_stage(l, h + 4, bq, sq_t)

                # ---------- Wo + residual ----------
                nc.scalar.activation(sq11[:], eps_t[0:1, 0:1], AF.Square)
                for cs in (0, CH):
                    for mp in range(KD // 2):
                        ps = psA.tile([P, 2, CH], f32, tag="a")
                        for bank in range(2):
                            m = 2 * mp + bank
                            for kb in range(NH):
                                nc.tensor.matmul(ps[:, bank, :],
                                                 wot[:, kb, m * P:(m + 1) * P],
                                                 aout[:, kb, cs:cs + CH],
                                                 start=(kb == 0), stop=(kb == NH - 1))
                        for bank in range(2):
                            m = 2 * mp + bank
                            nc.vector.scalar_tensor_tensor(
                                x[:, m, cs:cs + CH], ps[:, bank, :],
                                bo_t[:, m:m + 1], x[:, m, cs:cs + CH],
                                ALU.add, ALU.add)

                # ---------- LN2 + FFN + residual ----------
                layer_norm([0, CH])
                nc.scalar.activation(sq11[:], eps_t[0:1, 0:1], AF.Gelu)
                for cs in (0, CH):
                    ht = hp.tile([P, HB, CH], bf, tag="h")
                    for hp2 in range(HB // 2):
                        w1t = w1p.tile([P, KD, 2 * P], bf, tag="w1")
                        nc.sync.dma_start(
                            w1t[:], w1[l, :, hp2 * 2 * P:(hp2 + 1) * 2 * P].rearrange(
                                "(kt p) c -> p kt c", p=P))
                        ph = psA.tile([P, 2, CH], f32, tag="a")
                        for bank in range(2):
                            for kt in range(KD):
                                nc.tensor.matmul(ph[:, bank, :],
                                                 w1t[:, kt, bank * P:(bank + 1) * P],
                                                 xln[:, kt, cs:cs + CH],
                                                 start=(kt == 0), stop=(kt == KD - 1))
                        for bank in range(2):
                            hb = 2 * hp2 + bank
                            nc.scalar.activation(ht[:, hb, :], ph[:, bank, :], AF.Gelu,
                                                 bias=b1_t[:, hb:hb + 1],
                                                 scale=s1_t[:, hb:hb + 1])
                    # W2: 6 output blocks in parallel chains (2 psA pairs + 1 "v")
                    pa0 = psA.tile([P, 2, CH], f32, tag="a")
                    pa1 = psA.tile([P, 2, CH], f32, tag="a")
                    pa2 = psA.tile([P, 2, CH], f32, tag="v", bufs=1)
                    chains = [pa0[:, 0, :], pa0[:, 1, :], pa1[:, 0, :], pa1[:, 1, :],
                              pa2[:, 0, :], pa2[:, 1, :]]
                    for kb in range(HB):
                        w2t = w2p.tile([P, D], bf, tag="w2")
                        nc.sync.dma_start(w2t[:], w2[l, kb * P:(kb + 1) * P, :])
                        for m in range(KD):
                            nc.tensor.matmul(chains[m], w2t[:, m * P:(m + 1) * P],
                                             ht[:, kb, :],
                                             start=(kb == 0), stop=(kb == HB - 1))
                    for m in range(KD):
                        nc.vector.scalar_tensor_tensor(
                            x[:, m, cs:cs + CH], chains[m],
                            b2_t[:, m:m + 1], x[:, m, cs:cs + CH],
                            ALU.add, ALU.add)

            # ---------- final LN + decoder head ----------
            # final LN writes bf16 into the (now free) aout tile
            def to_aout(kt, cs, xc, pr_ap, eng):
                eng.tensor_mul(aout[:, kt, cs:cs + CH], xc[:], pr_ap)

            layer_norm([0, CH], to_aout)

            bd_t = biasp.tile([P, KD], f32, tag="bd")
            nc.sync.dma_start(bd_t[:], bdec[:])
            for ci in range(2):
                yT = yp.tile([P, KD, CH], f16, tag="yT")
                for mp in range(KD // 2):
                    ps = psA.tile([P, 2, CH], f32, tag="a")
                    for bank in range(2):
                        m = 2 * mp + bank
                        wdm = w1p.tile([P, KD, P], bf, tag="w1")
                        nc.sync.dma_start(
                            wdm[:], wdec[:, m * P:(m + 1) * P].rearrange(
                                "(kt p) c -> p kt c", p=P))
                        for kt in range(KD):
                            nc.tensor.matmul(ps[:, bank, :],
                                             wdm[:, kt, :],
                                             aout[:, kt, ci * CH:(ci + 1) * CH],
                                             start=(kt == 0), stop=(kt == KD - 1))
                    for bank in range(2):
                        m = 2 * mp + bank
                        nc.vector.tensor_scalar_add(yT[:, m, :], ps[:, bank, :],
                                                    bd_t[:, m:m + 1])
                for kt in range(KD):
                    nc.sync.dma_start(y[kt, :, ci * CH:(ci + 1) * CH], yT[:, kt, :])

    nc.compile()
    return nc


def _prep_weights(inputs):
    """Host-side weight folding/packing. Returns dict of shared arrays."""
    g1, be1 = inputs["gamma1"], inputs["beta1"]
    g2, be2 = inputs["gamma2"], inputs["beta2"]
    Wqkv, bqkv = inputs["Wqkv"], inputs["bqkv"]
    Wo, bo = inputs["Wo"], inputs["bo"]
    W1, b1 = inputs["W1"], inputs["b1"]
    W2, b2 = inputs["W2"], inputs["b2"]
    gn, gb = inputs["gn"], inputs["gb"]
    Wdec, bdec = inputs["Wdec"], inputs["bdec"]

    wqkv_a = np.zeros((DEPTH, D, NH * 3 * P), BF16)
    bqkv_a = np.zeros((DEPTH, NH * 3, P), np.float32)
    sqkv_a = np.ones((DEPTH, NH * 3, P), np.float32)
    wo_a = np.zeros((DEPTH, NH * P, D), np.float32)
    bwo_a = np.zeros((DEPTH, KD, P), np.float32)
    w1_a = np.zeros((DEPTH, D, HID), BF16)
    b1_a = np.zeros((DEPTH, HB, P), np.float32)
    s1_a = np.ones((DEPTH, HB, P), np.float32)
    w2_a = np.zeros((DEPTH, HID, D), BF16)
    b2_a = np.zeros((DEPTH, KD, P), np.float32)
    s2_a = np.ones((DEPTH, KD, P), np.float32)
    scale = 1.0 / np.sqrt(DH)
    for l in range(DEPTH):
        Wp = Wqkv[l] * g1[l][None, :]                  # fold gamma1
        bp = bqkv[l] + Wqkv[l] @ be1[l]                # fold beta1
        Wp = Wp.copy()
        bp = bp.copy()
        Wp[:D] *= scale                                # fold 1/sqrt(dh) into Q
        bp[:D] *= scale
        Wpq = Wp.astype(BF16)
        for h in range(NH):
            for c in range(3):                         # q,k,v
                rows = slice(c * D + h * DH, c * D + (h + 1) * DH)
                wqkv_a[l, :, (h * 3 + c) * P:(h * 3 + c) * P + DH] = Wpq[rows].T
                bqkv_a[l, h * 3 + c, :DH] = bp[rows]
            bqkv_a[l, h * 3 + 2, DH] = 1.0             # ones-row -> denominators
            wo_a[l, h * P:h * P + DH, :] = Wo[l][:, h * DH:(h + 1) * DH].T
        bwo_a[l] = bo[l].reshape(KD, P)
        W1f = W1[l] * g2[l][None, :]
        b1f = b1[l] + W1[l] @ be2[l]
        w1_a[l] = W1f.astype(BF16).T
        b1_a[l] = b1f.reshape(HB, P)
        w2_a[l] = W2[l].astype(BF16).T
        b2_a[l] = b2[l].reshape(KD, P)
    wdec_a = (Wdec * gn[None, :]).T
    bdec_a = (bdec + Wdec @ gb).reshape(KD, P)
    tp = lambda a: np.ascontiguousarray(a.transpose(0, 2, 1))
    return {
        "wqkv": wqkv_a, "bqkv": tp(bqkv_a), "sqkv": tp(sqkv_a),
        "wo": wo_a.astype(BF16), "bwo": tp(bwo_a),
        "w1": w1_a, "b1": tp(b1_a), "s1": tp(s1_a),
        "w2": w2_a, "b2": tp(b2_a), "s2": tp(s2_a),
        "wdec": wdec_a.astype(BF16), "bdec": np.ascontiguousarray(bdec_a.T),
    }


def kernel(**inputs):
    from concourse.bass_utils import run_bass_kernel_spmd

    inputs = {k: np.asarray(v) for k, v in inputs.items()}
    if "nc" not in _cache:
        _cache["nc"] = _build()
    nc = _cache["nc"]

    shared = _prep_weights(inputs)
    mask = inputs["mask"]
    vt = inputs["visible_tokens"].astype(np.float32)
    mt = inputs["mask_token"].astype(np.float32)

    # host-side scatter: x0[b, t] = vt[b, idx] if mask else mask_token
    nv = np.clip(np.cumsum(mask.astype(np.int64), axis=1) - 1, 0, N_VIS - 1)
    gathered = np.take_along_axis(vt, nv[..., None], axis=1)
    x0_full = np.where(mask[..., None], gathered, mt[None, None, :])  # (B,T,D)

    in_maps = []
    for core in range(8):
        b, s = core // 2, core % 2
        x0p = np.ascontiguousarray(
            x0_full[b][s * TQ:(s + 1) * TQ].T.astype(np.float16).reshape(KD, P, TQ))
        m = dict(shared)
        m["x0"] = x0p
        in_maps.append(m)

    res = run_bass_kernel_spmd(nc, in_maps, core_ids=list(range(8)),
                               **_cache.get("run_kwargs", {}))
    _cache["last_results"] = res

    out = np.zeros((B, T, D), np.float32)
    for core in range(8):
        b, s = core // 2, core % 2
        yv = res.results[core]["y"].reshape(D, TQ).astype(np.float32)
        out[b, s * TQ:(s + 1) * TQ] = yv.T
    return out


if __name__ == "__main__":
    print("building...")
    _build()
    print("built ok")


# revision 34
# speedup vs baseline: 1.1826x; 1.1826x over previous
"""Trainium2 Bass kernel for nn_DecoderHead (MAE-style decoder head), v2.

Strategy (8 NeuronCores): data-parallel over batch B=4 x 2-way token split
per batch. Each core computes Q/K/V only for its OWN 1024 tokens; K and V^T
halves are exchanged per head via 2-rank AllGather (pipelined 3 heads ahead
of the attention consumer, so the collective is fully hidden). Keys are laid
out in RANK ORDER (attention is key-permutation invariant), so the AllGather
result reads back with plain direct DMAs - no indirect gather needed.

On-device layout is feature-major (x^T: [D, T_own] with D on partitions).
Input scatter and output transpose are done host-side. Heads padded 96->128;
a ones-row injected in V (via bias) makes PV emit softmax denominators for
free. LN gamma/beta are folded into adjacent weights host-side. LN mean/rstd
and softmax-denominator broadcasts run on the (otherwise idle) GPSIMD engine
via partition_broadcast instead of PE matmuls. All GEMMs run in bf16 (fp8
e4m3 was measured at >2e-2 rel err - over the harness gate). PSUM is
organized as 2-bank pair tiles (3x "a" + 1x "v") so evictions are 1024 wide.
"""

import sys
import numpy as np

sys.path.insert(0, "/opt/trn_rl_repo")

import ml_dtypes

P = 128
B = 4
N_VIS = 512
T = 2048          # N_TOT
D = 768
KD = D // P       # 6
NH = 8
DH = 96
HID = 3072
HB = HID // P     # 24
DEPTH = 2
TQ = 1024         # own tokens per core
CH = 512          # token chunk
EPS = 1e-5

BF16 = ml_dtypes.bfloat16

_cache = {}


def _build():
    import concourse.bass as bass
    import concourse.mybir as mybir
    import concourse.tile as tile
    from concourse import bacc
    from concourse.masks import make_identity

    dt = mybir.dt
    nc = bacc.Bacc("TRN2", target_bir_lowering=False, debug=False, num_devices=8)

    x0 = nc.dram_tensor("x0", (KD, P, TQ), dt.float16, kind="ExternalInput").ap()
    wqkv = nc.dram_tensor("wqkv", (DEPTH, D, NH * 3 * P), dt.bfloat16, kind="ExternalInput").ap()
    bqkv = nc.dram_tensor("bqkv", (DEPTH, P, NH * 3), dt.float32, kind="ExternalInput").ap()
    sqkv = nc.dram_tensor("sqkv", (DEPTH, P, NH * 3), dt.float32, kind="ExternalInput").ap()
    wo = nc.dram_tensor("wo", (DEPTH, NH * P, D), dt.bfloat16, kind="ExternalInput").ap()
    bwo = nc.dram_tensor("bwo", (DEPTH, P, KD), dt.float32, kind="ExternalInput").ap()
    w1 = nc.dram_tensor("w1", (DEPTH, D, HID), dt.bfloat16, kind="ExternalInput").ap()
    b1 = nc.dram_tensor("b1", (DEPTH, P, HB), dt.float32, kind="ExternalInput").ap()
    s1 = nc.dram_tensor("s1", (DEPTH, P, HB), dt.float32, kind="ExternalInput").ap()
    w2 = nc.dram_tensor("w2", (DEPTH, HID, D), dt.bfloat16, kind="ExternalInput").ap()
    b2 = nc.dram_tensor("b2", (DEPTH, P, KD), dt.float32, kind="ExternalInput").ap()
    s2 = nc.dram_tensor("s2", (DEPTH, P, KD), dt.float32, kind="ExternalInput").ap()
    wdec = nc.dram_tensor("wdec", (D, D), dt.bfloat16, kind="ExternalInput").ap()
    bdec = nc.dram_tensor("bdec", (P, KD), dt.float32, kind="ExternalInput").ap()
    y = nc.dram_tensor("y", (KD, P, TQ), dt.float16, kind="ExternalOutput").ap()

    RG = [[0, 1], [2, 3], [4, 5], [6, 7]]

    with tile.TileContext(nc) as tc:
        from contextlib import ExitStack
        ctx = ExitStack()
        with ctx:
            const = ctx.enter_context(tc.tile_pool(name="const", bufs=1))
            xp = ctx.enter_context(tc.tile_pool(name="xp", bufs=1))
            xlnp = ctx.enter_context(tc.tile_pool(name="xlnp", bufs=1))
            kp = ctx.enter_context(tc.tile_pool(name="kp", bufs=4))
            vp = ctx.enter_context(tc.tile_pool(name="vp", bufs=4))
            qp = ctx.enter_context(tc.tile_pool(name="qp", bufs=2))
            kstp = ctx.enter_context(tc.tile_pool(name="kstp", bufs=2))
            vtmp = ctx.enter_context(tc.tile_pool(name="vtmp", bufs=2))
            vstp = ctx.enter_context(tc.tile_pool(name="vstp", bufs=2))
            ptp = ctx.enter_context(tc.tile_pool(name="ptp", bufs=2))
            aop = ctx.enter_context(tc.tile_pool(name="aop", bufs=1))
            hp = ctx.enter_context(tc.tile_pool(name="hp", bufs=1))
            yp = ctx.enter_context(tc.tile_pool(name="yp", bufs=1))
            wkvp = ctx.enter_context(tc.tile_pool(name="wkvp", bufs=2))
            wqp = ctx.enter_context(tc.tile_pool(name="wqp", bufs=2))
            wop = ctx.enter_context(tc.tile_pool(name="wop", bufs=1))
            w1p = ctx.enter_context(tc.tile_pool(name="w1p", bufs=5))
            w2p = ctx.enter_context(tc.tile_pool(name="w2p", bufs=4))
            scr = ctx.enter_context(tc.tile_pool(name="scr", bufs=2))
            xcp = ctx.enter_context(tc.tile_pool(name="xcp", bufs=2))
            statp = ctx.enter_context(tc.tile_pool(name="statp", bufs=1))
            biasp = ctx.enter_context(tc.tile_pool(name="biasp", bufs=1))

            drp = ctx.enter_context(tc.tile_pool(name="drp", bufs=1, space="DRAM"))
            # PSUM budget (8 banks): psA tag "a" 3x2-bank + tag "v" 1x2-bank
            psA = ctx.enter_context(tc.tile_pool(name="psA", bufs=3, space="PSUM"))

            f32 = dt.float32
            bf = dt.bfloat16
            f16 = dt.float16
            AF = mybir.ActivationFunctionType
            ALU = mybir.AluOpType

            ones_kx1b = const.tile([P, 1], bf, tag="ones_kx1b")
            nc.any.memset(ones_kx1b[:], 1.0 / D)
            ones_kx1h = const.tile([P, 1], f16, tag="ones_kx1h")
            nc.any.memset(ones_kx1h[:], 1.0 / D)
            sq11 = const.tile([1, 1], f32, tag="sq11")
            ident_bf = const.tile([P, P], bf, tag="ident_bf")
            make_identity(nc, ident_bf[:])
            eps_t = const.tile([1, 1], f32, tag="eps")
            nc.any.memset(eps_t[:], EPS)

            # persistent activations (own token half only)
            x = xp.tile([P, KD, TQ], f16, tag="x")
            xln = xlnp.tile([P, KD, TQ], bf, tag="xln")
            aout = aop.tile([P, NH, TQ], bf, tag="aout")

            # ---- PE warmup (runs during input DMA, releases HAM throttle) ----
            for wu in range(48):
                wt = psA.tile([P, 2, CH], bf, tag="a", name=f"wu{wu}")
                nc.tensor.transpose(wt[:, 0, 0:P], ident_bf[:], ident_bf[:])

            # ---- input DMA: x0^T own half ----
            for tb in range(2):
                for kt in range(KD):
                    nc.sync.dma_start(x[:, kt, tb * CH:(tb + 1) * CH],
                                      x0[kt, :, tb * CH:(tb + 1) * CH])

            _warm_ctr = [0]

            def layer_norm(chunks, out_fn=None):
                """LN over feature dim of x. Two passes: stats for ALL chunks
                first (PE ones-matmuls scaled by 1/D so PSUM holds mu/E2
                directly; sum chain interleaved with scalar-engine squares),
                then per-chunk postproc + gpsimd broadcast + DVE centering.
                Keep-warm transposes bridge the postproc PE idle window."""
                pss = {}
                for cs in chunks:
                    ps = psA.tile([P, 2, CH], f32, tag="a")
                    sqs = []
                    for kt in range(KD):
                        sq = scr.tile([P, CH], bf, tag="scrb")
                        nc.scalar.activation(sq[:], x[:, kt, cs:cs + CH], AF.Square)
                        sqs.append(sq)
                        nc.tensor.matmul(ps[0:1, 0, :], ones_kx1h[:], x[:, kt, cs:cs + CH],
                                         start=(kt == 0), stop=(kt == KD - 1))
                        if kt > 0:
                            nc.tensor.matmul(ps[0:1, 1, :], ones_kx1b[:], sqs[kt - 1][:],
                                             start=(kt == 1), stop=False)
                    nc.tensor.matmul(ps[0:1, 1, :], ones_kx1b[:], sqs[KD - 1][:],
                                     start=False, stop=True)
                    pss[cs] = ps
                # prefetch the Sqrt ACT table while PE finishes stats
                nc.scalar.activation(sq11[:], eps_t[0:1, 0:1], AF.Sqrt)
                # HAM keep-warm fillers through the (idle-during-LN) "v" pair;
                # the "a" ring stays free for stats/next-phase chains
                for wv in range(8):
                    wt2 = psA.tile([P, 2, CH], bf, tag="v", bufs=1,
                                   name=f"lnw{_warm_ctr[0]}_{wv}")
                    nc.tensor.transpose(wt2[:, 0, 0:P], ident_bf[:], ident_bf[:])
                _warm_ctr[0] += 1
                for cs in chunks:
                    ps = pss[cs]
                    st = statp.tile([1, 4 * CH], f32, tag="stats")
                    sg = lambda i: st[0:1, i * CH:(i + 1) * CH]
                    stb = statp.tile([1, 2 * CH], bf, tag="stb")
                    pms = xcp.tile([P, 2, CH], bf, tag="pms")
                    # mu path first so centering-sub can start early
                    nc.vector.tensor_copy(stb[0:1, 0:CH], ps[0:1, 0, :])
                    nc.gpsimd.partition_broadcast(pms[:, 0:1, :], stb[0:1, 0:CH])
                    nc.vector.tensor_mul(sg(0), stb[0:1, 0:CH], stb[0:1, 0:CH])  # mu^2
                    nc.vector.tensor_sub(sg(1), ps[0:1, 1, :], sg(0))          # var
                    nc.scalar.activation(sg(0), sg(1), AF.Sqrt,
                                         bias=eps_t[0:1, 0:1])
                    nc.vector.reciprocal_approx_fast(sg(3), sg(0))             # r
                    nc.vector.tensor_copy(stb[0:1, CH:2 * CH], sg(3))
                    nc.gpsimd.partition_broadcast(pms[:, 1:2, :], stb[0:1, CH:2 * CH])
                    for kt in range(KD):
                        xc = xcp.tile([P, CH], bf, tag="xc")
                        eng = nc.vector
                        eng.tensor_sub(xc[:], x[:, kt, cs:cs + CH], pms[:, 0, :])
                        if out_fn is None:
                            eng.tensor_mul(xln[:, kt, cs:cs + CH], xc[:],
                                           pms[:, 1, :])
                        else:
                            out_fn(kt, cs, xc, pms[:, 1, :], eng)

            def kv_stage(l, h, bq, sq_t):
                """K/V chains for own half, V^T transpose, AllGather both."""
                wkv = wkvp.tile([P, KD, 2 * P], bf, tag="wkv")
                nc.sync.dma_start(
                    wkv[:], wqkv[l, :, h * 3 * P + P:(h + 1) * 3 * P].rearrange(
                        "(kt p) c -> p kt c", p=P))
                kst = kstp.tile([P, TQ], bf, tag="kst")
                vtm = vtmp.tile([P, TQ], bf, tag="vtm")
                kvi = drp.tile([P, 2 * TQ], bf, tag=f"kvi{l}_{h}", name=f"kvi{l}_{h}")
                kvo = drp.tile([2 * P, 2 * TQ], bf, tag=f"kvo{l}_{h}",
                               name=f"kvo{l}_{h}")
                for m, dst in ((1, kst), (2, vtm)):
                    ps = psA.tile([P, 2, CH], f32, tag="a")
                    for bank in range(2):
                        for kt in range(KD):
                            nc.tensor.matmul(ps[:, bank, :],
                                             wkv[:, kt, (m - 1) * P:m * P],
                                             xln[:, kt, bank * CH:(bank + 1) * CH],
                                             start=(kt == 0), stop=(kt == KD - 1))
                    c = h * 3 + m
                    nc.vector.tensor_scalar(dst[:, :], ps[:, 0:2, :],
                                            sq_t[:, c:c + 1], bq[:, c:c + 1],
                                            ALU.mult, ALU.add)
                    if m == 1:
                        # stage K immediately, split across DMA queues
                        nc.sync.dma_start(kvi[:, 0:CH], kst[:, 0:CH])
                        nc.sync.dma_start(kvi[:, CH:TQ], kst[:, CH:TQ])
                vst = vstp.tile([P, 8, P], bf, tag="vst")
                for i in range(8):
                    pst = psA.tile([P, 2, CH], bf, tag="a")
                    nc.tensor.transpose(pst[:, 0, 0:P], vtm[:, i * P:(i + 1) * P],
                                        ident_bf[:])
                    nc.vector.tensor_copy(vst[:, i, :], pst[:, 0, 0:P])
                nc.sync.dma_start(kvi[:, TQ:TQ + CH], vst[:, 0:4, :])
                nc.sync.dma_start(kvi[:, TQ + CH:2 * TQ], vst[:, 4:8, :])
                nc.gpsimd.collective_compute(
                    "AllGather", mybir.AluOpType.bypass, replica_groups=RG,
                    ins=[kvi.opt()], outs=[kvo.opt()])
                # rank-ordered readback: key block r*8..r*8+7 = rank r's tokens
                ktile = kp.tile([P, T], bf, tag="k")
                vaug = vp.tile([P, T // P, P], bf, tag="vaug")
                for r in range(2):
                    nc.gpsimd.dma_start(ktile[:, r * TQ:(r + 1) * TQ],
                                        kvo[r * P:(r + 1) * P, 0:TQ])
                    nc.gpsimd.dma_start(vaug[:, r * 8:(r + 1) * 8, :],
                                        kvo[r * P:(r + 1) * P, TQ:2 * TQ])
                return ktile, vaug

            def attn_stage(l, h, ktile, vaug, bq, sq_t):
                """Q chain + scores^T -> exp -> PV + denom normalize."""
                wq = wqp.tile([P, KD, P], bf, tag="wq")
                nc.sync.dma_start(
                    wq[:], wqkv[l, :, h * 3 * P:h * 3 * P + P].rearrange(
                        "(kt p) c -> p kt c", p=P))
                qt = qp.tile([P, TQ], bf, tag="q")
                ps = psA.tile([P, 2, CH], f32, tag="a")
                for bank in range(2):
                    for kt in range(KD):
                        nc.tensor.matmul(ps[:, bank, :], wq[:, kt, :],
                                         xln[:, kt, bank * CH:(bank + 1) * CH],
                                         start=(kt == 0), stop=(kt == KD - 1))
                c = h * 3
                nc.vector.tensor_scalar(qt[:, :], ps[:, 0:2, :],
                                        sq_t[:, c:c + 1], bq[:, c:c + 1],
                                        ALU.mult, ALU.add)
                for cs in (0, CH):
                    pvc = psA.tile([P, 2, CH], f32, tag="v", bufs=1)
                    pv = pvc[:, 0, :]
                    nmm = 0
                    for half in range(2):
                        pt = ptp.tile([P, 8, CH], bf, tag="pt")
                        for tp in range(4):
                            tb = half * 8 + 2 * tp
                            sc = psA.tile([P, 2, CH], f32, tag="a")
                            for k2 in range(2):
                                nc.tensor.matmul(
                                    sc[:, k2, :],
                                    ktile[:, (tb + k2) * P:(tb + k2 + 1) * P],
                                    qt[:, cs:cs + CH],
                                    start=True, stop=True)
                            nc.scalar.activation(pt[:, 2 * tp:2 * tp + 2, :],
                                                 sc[:, 0:2, :], AF.Exp)
                            for k2 in range(2):
                                nc.tensor.matmul(pv, vaug[:, tb + k2, :],
                                                 pt[:, 2 * tp + k2, :],
                                                 start=(nmm == 0),
                                                 stop=(nmm == T // P - 1))
                                nmm += 1
                    # normalize by denominator (row 96 of pv)
                    dn = scr.tile([1, CH], f32, tag="scr")
                    nc.vector.tensor_copy(dn[:], pv[DH:DH + 1, :])
                    rc1 = statp.tile([1, CH], f32, tag="stb")
                    nc.vector.reciprocal_approx_fast(rc1[:], dn[:])
                    rcb = scr.tile([P, CH], f32, tag="scr")
                    nc.gpsimd.partition_broadcast(rcb[:], rc1[:])
                    nc.vector.tensor_mul(aout[:, h, cs:cs + CH], pv, rcb[:])

            for l in range(DEPTH):
                # ---------- LN1 (own half only) ----------
                layer_norm([0, CH])
                # prefetch Exp table on idle scalar before attention
                nc.scalar.activation(sq11[:], eps_t[0:1, 0:1], AF.Exp)

                # per-layer bias/scale tiles
                bq = biasp.tile([P, NH * 3], f32, tag="bq")
                nc.sync.dma_start(bq[:], bqkv[l])
                sq_t = biasp.tile([P, NH * 3], f32, tag="sq")
                nc.sync.dma_start(sq_t[:], sqkv[l])
                bo_t = biasp.tile([P, KD], f32, tag="bo")
                nc.sync.dma_start(bo_t[:], bwo[l])
                b1_t = biasp.tile([P, HB], f32, tag="b1")
                nc.sync.dma_start(b1_t[:], b1[l])
                s1_t = biasp.tile([P, HB], f32, tag="s1")
                nc.sync.dma_start(s1_t[:], s1[l])
                b2_t = biasp.tile([P, KD], f32, tag="b2")
                nc.sync.dma_start(b2_t[:], b2[l])
                s2_t = biasp.tile([P, KD], f32, tag="s2")
                nc.sync.dma_start(s2_t[:], s2[l])

                wot = wop.tile([P, NH, D], bf, tag="wo")
                nc.sync.dma_start(wot[:], wo[l].rearrange("(kb p) c -> p kb c", p=P))

                # ---------- attention: kv pipeline 3 ahead of consumer ----------
                kv_tiles = {}
                for h in range(3):
                    kv_tiles[h] = kv_stage(l, h, bq, sq_t)
                for h in range(NH):
                    attn_stage(l, h, *kv_tiles.pop(h), bq, sq_t)
                    if h + 3 < NH:
                        kv_tiles[h + 3] = kv_stage(l, h + 3, bq, sq_t)

                # ---------- Wo + residual ----------
                nc.scalar.activation(sq11[:], eps_t[0:1, 0:1], AF.Square)
                for cs in (0, CH):
                    for mp in range(KD // 2):
                        ps = psA.tile([P, 2, CH], f32, tag="a")
                        for bank in range(2):
                            m = 2 * mp + bank
                            for kb in range(NH):
                                nc.tensor.matmul(ps[:, bank, :],
                                                 wot[:, kb, m * P:(m + 1) * P],
                                                 aout[:, kb, cs:cs + CH],
                                                 start=(kb == 0), stop=(kb == NH - 1))
                        for bank in range(2):
                            m = 2 * mp + bank
                            nc.vector.scalar_tensor_tensor(
                                x[:, m, cs:cs + CH], ps[:, bank, :],
                                bo_t[:, m:m + 1], x[:, m, cs:cs + CH],
                                ALU.add, ALU.add)

                # ---------- LN2 + FFN + residual ----------
                layer_norm([0, CH])
                nc.scalar.activation(sq11[:], eps_t[0:1, 0:1], AF.Gelu)
                for cs in (0, CH):
                    ht = hp.tile([P, HB, CH], bf, tag="h")
                    for hp2 in range(HB // 2):
                        w1t = w1p.tile([P, KD, 2 * P], bf, tag="w1")
                        nc.sync.dma_start(
                            w1t[:], w1[l, :, hp2 * 2 * P:(hp2 + 1) * 2 * P].rearrange(
                                "(kt p) c -> p kt c", p=P))
                        ph = psA.tile([P, 2, CH], f32, tag="a")
                        for bank in range(2):
                            for kt in range(KD):
                                nc.tensor.matmul(ph[:, bank, :],
                                                 w1t[:, kt, bank * P:(bank + 1) * P],
                                                 xln[:, kt, cs:cs + CH],
                                                 start=(kt == 0), stop=(kt == KD - 1))
                        for bank in range(2):
                            hb = 2 * hp2 + bank
                            nc.scalar.activation(ht[:, hb, :], ph[:, bank, :], AF.Gelu,
                                                 bias=b1_t[:, hb:hb + 1],
                                                 scale=s1_t[:, hb:hb + 1])
                    # W2: 6 output blocks in parallel chains (2 psA pairs + 1 "v")
                    pa0 = psA.tile([P, 2, CH], f32, tag="a")
                    pa1 = psA.tile([P, 2, CH], f32, tag="a")
                    pa2 = psA.tile([P, 2, CH], f32, tag="v", bufs=1)
                    chains = [pa0[:, 0, :], pa0[:, 1, :], pa1[:, 0, :], pa1[:, 1, :],
                              pa2[:, 0, :], pa2[:, 1, :]]
                    for kb in range(HB):
                        w2t = w2p.tile([P, D], bf, tag="w2")
                        nc.sync.dma_start(w2t[:], w2[l, kb * P:(kb + 1) * P, :])
                        for m in range(KD):
                            nc.tensor.matmul(chains[m], w2t[:, m * P:(m + 1) * P],
                                             ht[:, kb, :],
                                             start=(kb == 0), stop=(kb == HB - 1))
                    for m in range(KD):
                        nc.vector.scalar_tensor_tensor(
                            x[:, m, cs:cs + CH], chains[m],
                            b2_t[:, m:m + 1], x[:, m, cs:cs + CH],
                            ALU.add, ALU.add)

            # ---------- final LN + decoder head ----------
            # final LN writes bf16 into the (now free) aout tile
            def to_aout(kt, cs, xc, pr_ap, eng):
                eng.tensor_mul(aout[:, kt, cs:cs + CH], xc[:], pr_ap)

            layer_norm([0, CH], to_aout)

            bd_t = biasp.tile([P, KD], f32, tag="bd")
            nc.sync.dma_start(bd_t[:], bdec[:])
            for ci in range(2):
                yT = yp.tile([P, KD, CH], f16, tag="yT")
                for mp in range(KD // 2):
                    ps = psA.tile([P, 2, CH], f32, tag="a")
                    for bank in range(2):
                        m = 2 * mp + bank
                        wdm = w1p.tile([P, KD, P], bf, tag="w1")
                        nc.sync.dma_start(
                            wdm[:], wdec[:, m * P:(m + 1) * P].rearrange(
                                "(kt p) c -> p kt c", p=P))
                        for kt in range(KD):
                            nc.tensor.matmul(ps[:, bank, :],
                                             wdm[:, kt, :],
                                             aout[:, kt, ci * CH:(ci + 1) * CH],
                                             start=(kt == 0), stop=(kt == KD - 1))
                    for bank in range(2):
                        m = 2 * mp + bank
                        nc.vector.tensor_scalar_add(yT[:, m, :], ps[:, bank, :],
                                                    bd_t[:, m:m + 1])
                for kt in range(KD):
                    nc.sync.dma_start(y[kt, :, ci * CH:(ci + 1) * CH], yT[:, kt, :])

    nc.compile()
    return nc


def _prep_weights(inputs):
    """Host-side weight folding/packing. Returns dict of shared arrays."""
    g1, be1 = inputs["gamma1"], inputs["beta1"]
    g2, be2 = inputs["gamma2"], inputs["beta2"]
    Wqkv, bqkv = inputs["Wqkv"], inputs["bqkv"]
    Wo, bo = inputs["Wo"], inputs["bo"]
    W1, b1 = inputs["W1"], inputs["b1"]
    W2, b2 = inputs["W2"], inputs["b2"]
    gn, gb = inputs["gn"], inputs["gb"]
    Wdec, bdec = inputs["Wdec"], inputs["bdec"]

    wqkv_a = np.zeros((DEPTH, D, NH * 3 * P), BF16)
    bqkv_a = np.zeros((DEPTH, NH * 3, P), np.float32)
    sqkv_a = np.ones((DEPTH, NH * 3, P), np.float32)
    wo_a = np.zeros((DEPTH, NH * P, D), np.float32)
    bwo_a = np.zeros((DEPTH, KD, P), np.float32)
    w1_a = np.zeros((DEPTH, D, HID), BF16)
    b1_a = np.zeros((DEPTH, HB, P), np.float32)
    s1_a = np.ones((DEPTH, HB, P), np.float32)
    w2_a = np.zeros((DEPTH, HID, D), BF16)
    b2_a = np.zeros((DEPTH, KD, P), np.float32)
    s2_a = np.ones((DEPTH, KD, P), np.float32)
    scale = 1.0 / np.sqrt(DH)
    for l in range(DEPTH):
        Wp = Wqkv[l] * g1[l][None, :]                  # fold gamma1
        bp = bqkv[l] + Wqkv[l] @ be1[l]                # fold beta1
        Wp = Wp.copy()
        bp = bp.copy()
        Wp[:D] *= scale                                # fold 1/sqrt(dh) into Q
        bp[:D] *= scale
        Wpq = Wp.astype(BF16)
        for h in range(NH):
            for c in range(3):                         # q,k,v
                rows = slice(c * D + h * DH, c * D + (h + 1) * DH)
                wqkv_a[l, :, (h * 3 + c) * P:(h * 3 + c) * P + DH] = Wpq[rows].T
                bqkv_a[l, h * 3 + c, :DH] = bp[rows]
            bqkv_a[l, h * 3 + 2, DH] = 1.0             # ones-row -> denominators
            wo_a[l, h * P:h * P + DH, :] = Wo[l][:, h * DH:(h + 1) * DH].T
        bwo_a[l] = bo[l].reshape(KD, P)
        W1f = W1[l] * g2[l][None, :]
        b1f = b1[l] + W1[l] @ be2[l]
        w1_a[l] = W1f.astype(BF16).T
        b1_a[l] = b1f.reshape(HB, P)
        w2_a[l] = W2[l].astype(BF16).T
        b2_a[l] = b2[l].reshape(KD, P)
    wdec_a = (Wdec * gn[None, :]).T
    bdec_a = (bdec + Wdec @ gb).reshape(KD, P)
    tp = lambda a: np.ascontiguousarray(a.transpose(0, 2, 1))
    return {
        "wqkv": wqkv_a, "bqkv": tp(bqkv_a), "sqkv": tp(sqkv_a),
        "wo": wo_a.astype(BF16), "bwo": tp(bwo_a),
        "w1": w1_a, "b1": tp(b1_a), "s1": tp(s1_a),
        "w2": w2_a, "b2": tp(b2_a), "s2": tp(s2_a),
        "wdec": wdec_a.astype(BF16), "bdec": np.ascontiguousarray(bdec_a.T),
    }


def kernel(**inputs):
    from concourse.bass_utils import run_bass_kernel_spmd

    inputs = {k: np.asarray(v) for k, v in inputs.items()}
    if "nc" not in _cache:
        _cache["nc"] = _build()
    nc = _cache["nc"]

    shared = _prep_weights(inputs)
    mask = inputs["mask"]
    vt = inputs["visible_tokens"].astype(np.float32)
    mt = inputs["mask_token"].astype(np.float32)

    # host-side scatter: x0[b, t] = vt[b, idx] if mask else mask_token
    nv = np.clip(np.cumsum(mask.astype(np.int64), axis=1) - 1, 0, N_VIS - 1)
    gathered = np.take_along_axis(vt, nv[..., None], axis=1)
    x0_full = np.where(mask[..., None], gathered, mt[None, None, :])  # (B,T,D)

    in_maps = []
    for core in range(8):
        b, s = core // 2, core % 2
        x0p = np.ascontiguousarray(
            x0_full[b][s * TQ:(s + 1) * TQ].T.astype(np.float16).reshape(KD, P, TQ))
        m = dict(shared)
        m["x0"] = x0p
        in_maps.append(m)

    res = run_bass_kernel_spmd(nc, in_maps, core_ids=list(range(8)),
                               **_cache.get("run_kwargs", {}))
    _cache["last_results"] = res

    out = np.zeros((B, T, D), np.float32)
    for core in range(8):
        b, s = core // 2, core % 2
        yv = res.results[core]["y"].reshape(D, TQ).astype(np.float32)
        out[b, s * TQ:(s + 1) * TQ] = yv.T
    return out


if __name__ == "__main__":
    print("building...")
    _build()
    print("built ok")
